# revision 8
# baseline (speedup 1.0000x reference)
"""BitLinear (absmean ternary quantized linear) on 8 TRN2 NeuronCores.

out[b,t,o] = sum_i x[b,t,i] * (clip(round(W[o,i]/delta), -1, 1) * delta) + bias[o]
delta = mean(|W|) + 1e-8  over the FULL weight.

Sharding: tensor-parallel over OUT rows (11008 / 8 = 1376 rows per core).
x is replicated. delta partial abs-sums are AllGathered across the 8 cores.
Host passes each core its weight shard transposed ([IN, OUT_SH], contiguous)
so the contraction dim lands on SBUF partitions; x stays in natural layout
and is transposed on-device by the TensorEngine; host concatenates the 8
output shards.

Quantization without round() (not available on any engine):
  q = clip(round(w/d),-1,1) = 1[w >= d/2] - 1[w <= -d/2]      (a.e.)
    = (sign(w - d/2) + sign(w + d/2)) / 2                      (a.e.)
The matmul distributes over the two threshold maps, so each map (exact in
bf16) feeds its own matmul stream:
  psum += xbf @ a.T + xbf @ (-b).T        [DVE/GpSimd threshold method]
  psum += (x/2) @ s1.T + (x/2) @ s2.T     [ACT sign method]
and the epilogue applies out = delta * psum (+ bias via a K=1 PSUM-init
matmul of bias/delta). k-tiles are split across ACT/DVE/GPSIMD to balance
engine time. PE is kept HAM-warm through pass A and the collective gap
with cheap chained dummy matmuls.
"""

import numpy as np

B, T, IN, OUT = 8, 16, 4096, 11008
M = B * T               # 128 tokens
CORES = 8
OUT_SH = OUT // CORES   # 1376
KT = IN // 128          # 32 k-tiles
N_TOTAL_W = OUT * IN    # 45088768
EPS = 1e-8

RESIDENT = 23           # k-tiles kept SBUF-resident between pass A and B
N_STREAM = KT - RESIDENT  # first N_STREAM k-tiles stream through wstream slots
WS_BUFS = 4             # wstream slots
N_ACT, N_GPS = 13, 0    # map lanes: ACT sign / GPSIMD threshold; rest DVE
COL_SLICES = [(0, 512), (512, 1024), (1024, OUT_SH)]
GAP_CHAIN = 12          # PE<->DVE ping-pong links bridging the collective gap
WARM_BURST = 32         # dense N=256 bf16 matmuls to flip HAM warm before pass B


def _lane_assignment():
    quotas = {"A": N_ACT, "G": N_GPS, "D": KT - N_ACT - N_GPS}
    used = {k: 0 for k in quotas}
    lanes = []
    for k in range(KT):
        lane = max(quotas, key=lambda l: quotas[l] * (k + 1) / KT - used[l])
        used[lane] += 1
        lanes.append(lane)
    return lanes


LANES = _lane_assignment()
ACT_SET = {k for k, l in enumerate(LANES) if l == "A"}
XH_IDX = {k: i for i, k in enumerate(sorted(ACT_SET))}

_CACHE = {}


def _build():
    from concourse import bass, bacc, tile, mybir

    f32 = mybir.dt.float32
    bf16 = mybir.dt.bfloat16
    AF = mybir.ActivationFunctionType
    ALU = mybir.AluOpType

    nc = bacc.Bacc("TRN2", target_bir_lowering=False, debug=False, num_devices=CORES)

    wt_d = nc.dram_tensor("wt", [IN, OUT_SH], f32, kind="ExternalInput")
    xn_d = nc.dram_tensor("xn", [M, IN], f32, kind="ExternalInput")
    bias_d = nc.dram_tensor("bias", [1, OUT_SH], f32, kind="ExternalInput")
    out_d = nc.dram_tensor("out", [M, OUT_SH], f32, kind="ExternalOutput")
    ident_d = nc.inline_tensor(np.eye(128, dtype=np.float32), name="ident")

    with tile.TileContext(nc) as tc:
        with (
            tc.tile_pool(name="wres", bufs=RESIDENT) as wres,
            tc.tile_pool(name="wstream", bufs=WS_BUFS) as wstream,
            tc.tile_pool(name="xp", bufs=1) as xp,
            tc.tile_pool(name="bp", bufs=1) as bp,
            tc.tile_pool(name="cons", bufs=1) as cons,
            tc.tile_pool(name="stat", bufs=1) as stat,
            tc.tile_pool(name="maps", bufs=3) as maps,
            tc.tile_pool(name="op", bufs=1) as op,
            tc.tile_pool(name="dram", bufs=1, space="DRAM") as dram,
            tc.tile_pool(name="psmall", bufs=1, space="PSUM") as psmall,
            tc.tile_pool(name="pjunk", bufs=1, space="PSUM") as pjunk,
            tc.tile_pool(name="ptr", bufs=2, space="PSUM") as ptr,
            tc.tile_pool(name="pout", bufs=1, space="PSUM") as pout,
        ):
            # ---- weight DMAs first: they are the memory roofline ----
            w_tiles = {}
            w_all = []
            for k in range(KT):
                if k >= N_STREAM:
                    wk = wres.tile([128, OUT_SH], f32, tag="w")
                    w_tiles[k] = wk
                else:
                    wk = wstream.tile([128, OUT_SH], f32, tag="ws")
                nc.sync.dma_start(out=wk[:], in_=wt_d[128 * k : 128 * (k + 1), :])
                w_all.append(wk)
                if k == 5:
                    xnat = xp.tile([128, KT, M], f32)
                    nc.sync.dma_start(
                        out=xnat[:],
                        in_=xn_d[:].rearrange("p (t c) -> p t c", c=128),
                    )
                    bias_sb = bp.tile([1, OUT_SH], f32)
                    nc.sync.dma_start(out=bias_sb[:], in_=bias_d[:])

            # ---- constants / small tiles ----
            ones_col = cons.tile([128, 1], f32)
            ones_row = cons.tile([1, 128], f32)
            nc.gpsimd.memset(ones_col[:], 1.0)
            nc.gpsimd.memset(ones_row[:], 1.0)
            id_sb = cons.tile([128, 128], f32)
            nc.sync.dma_start(out=id_sb[:], in_=ident_d[:])
            ones2d = cons.tile([128, 128], f32)
            nc.gpsimd.memset(ones2d[:], 1.0)
            ones_row_bf = cons.tile([1, 128], bf16)
            nc.gpsimd.memset(ones_row_bf[:], 1.0)
            jrow_bf = cons.tile([1, 256], bf16)
            nc.gpsimd.memset(jrow_bf[:], 1.0)
            warm = cons.tile([128, 1], f32)
            # pre-load the ACT table set containing Sign while DMAs run
            nc.scalar.activation(warm[:], ones_col[:], AF.Sign)

            partials = stat.tile([128, KT], f32)
            sumP = stat.tile([128, 1], f32)
            s_sb = stat.tile([1, 8], f32)
            gath = stat.tile([8, 8], f32)
            d_sb = stat.tile([1, 1], f32)
            rd_sb = stat.tile([1, 1], f32)
            delta_bc = stat.tile([128, 1], f32)
            th = stat.tile([128, 1], f32)       # +delta/2
            nth = stat.tile([128, 1], f32)      # -delta/2
            junk_sb = stat.tile([128, 1], f32)
            wjunk = stat.tile([8, 8], f32)

            # early dummy collective: wakes ncfw so the real one starts fast
            ccw_in = dram.tile([1, 8], f32)
            ccw_out = dram.tile([8, 8], f32, addr_space="Shared")
            nc.gpsimd.dma_start(out=ccw_in[:], in_=ones_row[0:1, 0:8])
            nc.gpsimd.collective_compute(
                "AllGather",
                ALU.bypass,
                replica_groups=[list(range(CORES))],
                ins=[ccw_in[:].opt()],
                outs=[ccw_out[:].opt()],
            )
            nc.gpsimd.dma_start(out=wjunk[:], in_=ccw_out[:])

            psum_out = pout.tile([M, OUT_SH], f32)
            junk_ps = pjunk.tile([128, 512], f32)

            # ---- pass A: abs-sum each weight tile as it lands ----
            for k in range(KT):
                nc.vector.tensor_reduce(
                    partials[:, k : k + 1],
                    w_all[k][:],
                    axis=mybir.AxisListType.X,
                    op=ALU.add,
                    apply_absolute_value=True,
                )
                # PE warm-keeper: tiny matmul chained on this tile's partial
                nc.tensor.matmul(junk_ps[:, 0:1], ones_row[:], partials[0:1, k : k + 1])

            # ---- delta: local sum -> AllGather -> total -> broadcast ----
            nc.vector.tensor_reduce(
                sumP[:], partials[:], axis=mybir.AxisListType.X, op=ALU.add
            )
            ps1 = psmall.tile([1, 1], f32, tag="ps1")
            nc.tensor.matmul(ps1[:], sumP[:], ones_col[:])  # sum over partitions
            nc.gpsimd.memset(s_sb[:], 0.0)
            nc.vector.tensor_copy(s_sb[0:1, 0:1], ps1[:])

            cc_in = dram.tile([1, 8], f32)
            cc_out = dram.tile([8, 8], f32, addr_space="Shared")
            nc.gpsimd.dma_start(out=cc_in[:], in_=s_sb[:])
            nc.gpsimd.collective_compute(
                "AllGather",
                ALU.bypass,
                replica_groups=[list(range(CORES))],
                ins=[cc_in[:].opt()],
                outs=[cc_out[:].opt()],
            )
            nc.gpsimd.dma_start(out=gath[:], in_=cc_out[:])

            # ---- x transpose on PE (after pass-A loop so the DVE FIFO's
            # reduces aren't blocked behind psum->sbuf copies) ----
            xbf = xp.tile([128, KT, M], bf16)   # xT bf16 (DVE/GPS tiles)
            xh = xp.tile([128, N_ACT, M], bf16) # xT/2 bf16 (ACT tiles)
            for k in range(KT):
                pst = ptr.tile([128, M], f32, tag="tr")
                nc.tensor.transpose(pst[:], xnat[:, k, :], id_sb[:])
                if k in ACT_SET:
                    nc.vector.tensor_scalar_mul(xh[:, XH_IDX[k], :], pst[:], 0.5)
                else:
                    nc.vector.tensor_copy(xbf[:, k, :], pst[:])

            # PE warm-keeper chain across the collective gap: PE <-> DVE
            # ping-pong; each link's latency spaces the matmuls out in time.
            for _ in range(GAP_CHAIN):
                nc.vector.tensor_copy(junk_sb[:], junk_ps[:, 0:1])
                nc.tensor.matmul(junk_ps[:, 0:1], ones_row[:], junk_sb[0:1, 0:1])

            # S summed over cores AND broadcast to 128 partitions in ONE mm:
            # ones2d[0:8,:].T @ gath[0:8,0:1] -> [128,1] of S_total
            psb = psmall.tile([128, 1], f32, tag="psb")
            nc.tensor.matmul(psb[:], ones2d[0:8, :], gath[0:8, 0:1])
            # thresholds straight from PSUM: th = S*(0.5/N) + eps/2 = delta/2
            nc.vector.tensor_scalar(
                th[:], psb[:], 0.5 / N_TOTAL_W, EPS / 2, op0=ALU.mult, op1=ALU.add
            )
            nc.vector.tensor_scalar(
                nth[:], psb[:], -0.5 / N_TOTAL_W, -EPS / 2, op0=ALU.mult, op1=ALU.add
            )
            # off-critical-path: delta for the epilogue scale and 1/delta
            nc.vector.tensor_scalar(
                delta_bc[:], psb[:], 1.0 / N_TOTAL_W, EPS, op0=ALU.mult, op1=ALU.add
            )
            nc.vector.tensor_scalar(
                d_sb[:], psb[0:1, 0:1], 1.0 / N_TOTAL_W, EPS, op0=ALU.mult, op1=ALU.add
            )
            nc.vector.reciprocal(rd_sb[:], d_sb[:])

            # dense burst: flip HAM to 8/8 right before the real matmuls
            for _ in range(WARM_BURST):
                nc.tensor.matmul(junk_ps[:, 0:256], ones_row_bf[:], jrow_bf[:])

            # bias/delta into PSUM: ones[1,128].T @ (bias/delta) broadcasts
            nc.vector.tensor_scalar(
                bias_sb[:], bias_sb[:], rd_sb[:], None, op0=ALU.mult
            )
            for c0, c1 in COL_SLICES:
                nc.tensor.matmul(
                    psum_out[:, c0:c1],
                    ones_row[:],
                    bias_sb[:, c0:c1],
                    start=True,
                    stop=False,
                )

            # ---- pass B: streamed re-DMAs issued upfront (so the scheduler
            # fires them during the collective gap); first WS_BUFS streamed
            # tiles consumed first, the rest spread between resident tiles ----
            streamed = [k for k in range(KT) if k not in w_tiles]
            resident = [k for k in range(KT) if k in w_tiles]
            for k in streamed:
                wk = wstream.tile([128, OUT_SH], f32, tag="ws")
                nc.sync.dma_start(out=wk[:], in_=wt_d[128 * k : 128 * (k + 1), :])
                w_tiles[k] = wk
            pass_b_order = list(streamed[:WS_BUFS])
            rest = list(resident)
            for i, k in enumerate(streamed[WS_BUFS:]):
                pass_b_order.extend(rest[: 4 if i == 0 else 2])
                rest = rest[4 if i == 0 else 2 :]
                pass_b_order.append(k)
            pass_b_order.extend(rest)
            assert sorted(pass_b_order) == list(range(KT))
            for ki, k in enumerate(pass_b_order):
                wk = w_tiles[k]
                mA = maps.tile([128, OUT_SH], bf16, tag="mA")
                mB = maps.tile([128, OUT_SH], bf16, tag="mB")
                lane = LANES[k]
                if lane == "A":
                    # sign method on ACT; contributes 2q, x carries the 1/2
                    nc.scalar.activation(mA[:], wk[:], AF.Sign, bias=nth[:])
                    nc.scalar.activation(mB[:], wk[:], AF.Sign, bias=th[:])
                    xa = xh[:, XH_IDX[k], :]
                else:
                    # threshold method: q = a - b; minus folded into the map
                    eng = nc.vector if lane == "D" else nc.gpsimd
                    eng.tensor_scalar(mA[:], wk[:], th[:], None, op0=ALU.is_ge)
                    eng.tensor_scalar(
                        mB[:], wk[:], nth[:], -1.0, op0=ALU.is_le, op1=ALU.mult
                    )
                    xa = xbf[:, k, :]
                last = ki == KT - 1
                for c0, c1 in COL_SLICES:
                    nc.tensor.matmul(
                        psum_out[:, c0:c1], xa[:], mA[:, c0:c1], start=False, stop=False
                    )
                for c0, c1 in COL_SLICES:
                    nc.tensor.matmul(
                        psum_out[:, c0:c1], xa[:], mB[:, c0:c1], start=False, stop=last
                    )

            # epilogue: out = delta * psum  (bias already in as bias/delta)
            out_sb = op.tile([M, OUT_SH], f32)
            nc.scalar.activation(
                out_sb[:], psum_out[:], AF.Identity, bias=0.0, scale=delta_bc[:]
            )
            nc.sync.dma_start(out=out_d[:], in_=out_sb[:])

    nc.compile()
    return nc


def _get_nc():
    if "nc" not in _CACHE:
        _CACHE["nc"] = _build()
    return _CACHE["nc"]


def _run(x, weight, bias, **spmd_kwargs):
    from concourse.bass_utils import run_bass_kernel_spmd

    x = np.ascontiguousarray(np.asarray(x), dtype=np.float32)
    weight = np.ascontiguousarray(np.asarray(weight), dtype=np.float32)
    bias = np.ascontiguousarray(np.asarray(bias), dtype=np.float32)

    xn = x.reshape(M, IN)
    in_maps = []
    for c in range(CORES):
        rows = slice(c * OUT_SH, (c + 1) * OUT_SH)
        in_maps.append(
            {
                "xn": xn,
                "wt": np.ascontiguousarray(weight[rows].T),  # [IN, OUT_SH]
                "bias": bias[rows].reshape(1, OUT_SH),
            }
        )
    nc = _get_nc()
    res = run_bass_kernel_spmd(nc, in_maps, core_ids=list(range(CORES)), **spmd_kwargs)
    out = np.concatenate([res.results[c]["out"] for c in range(CORES)], axis=1)
    return out.reshape(B, T, OUT).astype(np.float32), res


def kernel(x, weight, bias):
    out, _ = _run(x, weight, bias)
    return out
